# revision 1
# baseline (speedup 1.0000x reference)
"""Trainium2 Bass kernel for masked multi-head attention.

Problem: B=4, S=2048, D=768, H=12 (head_dim=64), boolean prune mask per
head, softmax over keys, out-projection.

Sharding (8 cores): data-parallel over batch (4) x tensor-parallel over
head halves (2 x 6 heads).  Core c handles batch c//2 and heads
(c%2)*6 .. (c%2)*6+5.  Each core computes its 6 heads' QKV projections,
attention, and the partial out-projection (row-parallel slice of out_w).
The host sums the two partials per batch and adds out_b (the standard
tensor-parallel reduce, done during unshard).

On-chip layout choices:
  * Activations are kept feature-major ("transposed"): hsT [769, 2048]
    (row 768 = ones for the affine/bias trick), qT/kT [384, 2048].
  * Scores are computed directly transposed: S_T[k, q] = kT.T-slice @ qT,
    so P.T is exactly the rhs the ctx matmul needs -> no transposes.
  * The mask is pre-transposed to [k, q] per head on the host (bf16 0/1),
    applied multiplicatively after exp (equivalent to -inf before
    softmax, and safe: |scores/8| < ~2 so no overflow without max-sub).
  * V carries an appended ones column per head (wvT has 6x65 columns)
    so row 64 of each ctx PSUM accumulates the softmax denominators.
  * Normalization: denominators gathered per head via a casting SWDGE
    DMA, one batched DVE reciprocal at the end, broadcast across each
    head's 64 partitions with a selector matmul, then DVE multiplies.
  * All matmul inputs bf16 (PSUM accumulates f32); output stored bf16.
  * Scores/ctx stationaries zero-padded to K=128 / M=128 (half-active
    PE arrays made the HAM clock gate hold the PE at 1.2 GHz), plus a
    PE warm-up spin during the initial DMAs.
"""

import os
import sys
import math

import numpy as np

try:
    import concourse.bass as bass
except ImportError:  # pragma: no cover - path fallback for fresh dirs
    for _p in ("/opt/trn_rl_repo", "/root/.axon_site/_ro/trn_rl_repo"):
        if os.path.isdir(_p) and _p not in sys.path:
            sys.path.insert(0, _p)
    import concourse.bass as bass

import ml_dtypes
import concourse.mybir as mybir
from concourse import bacc
from concourse.tile import TileContext
from concourse.bass_utils import run_bass_kernel_spmd

BF16 = ml_dtypes.bfloat16
F32 = mybir.dt.float32
BBF = mybir.dt.bfloat16

B, S, D, H = 4, 2048, 768, 12
HD = 64          # head dim
HPC = 6          # heads per core
FPC = HPC * HD   # features per core (384)
NCORES = 8
KT = S // 128    # 16 key tiles
ST = S // 128    # 16 seq tiles

_CACHE = {}
_last_result = None


def _build_bass():
    nc = bacc.Bacc()

    hsT = nc.declare_dram_parameter("hsT", [D + 1, S], BBF, isOutput=False)
    wqT = nc.declare_dram_parameter("wqT", [D + 1, FPC], BBF, isOutput=False)
    wkT = nc.declare_dram_parameter("wkT", [D + 1, FPC], BBF, isOutput=False)
    wvT = nc.declare_dram_parameter("wvT", [D + 1, HPC * (HD + 1)], BBF, isOutput=False)
    owT = nc.declare_dram_parameter("owT", [FPC, D], BBF, isOutput=False)
    selp = nc.declare_dram_parameter("selp", [128, FPC], BBF, isOutput=False)
    mT = nc.declare_dram_parameter("mT", [HPC, KT, 128, S], BBF, isOutput=False)
    out = nc.declare_dram_parameter("out", [S, D], BBF, isOutput=True)

    EXP = mybir.ActivationFunctionType.Exp
    LN = mybir.ActivationFunctionType.Ln
    MULT = mybir.AluOpType.mult

    with TileContext(nc) as tc, \
            tc.tile_pool(name="persist", bufs=1) as pp, \
            tc.tile_pool(name="maskp", bufs=3) as mask_pool, \
            tc.tile_pool(name="pbuf", bufs=3) as p_pool, \
            tc.tile_pool(name="obuf", bufs=2) as o_pool, \
            tc.tile_pool(name="pswork", bufs=2, space="PSUM") as ps_pool, \
            tc.tile_pool(name="psctx", bufs=1, space="PSUM") as ctx_pool:

        # ---------------- persistent SBUF tensors + input DMAs ----------
        hsT_sb = [pp.tile([128, S], BBF, name=f"hsT{c}", tag=f"hsT{c}")
                  for c in range(6)]
        # all-ones row: content is uniform, so a [1, 512] tile serves every
        # 512-wide rhs slice and every 128-wide lhsT slice
        ones_sb = pp.tile([1, 512], BBF, name="ones_row", tag="ones_row")
        for c in range(6):
            eng = nc.sync if c % 2 == 0 else nc.scalar
            eng.dma_start(out=hsT_sb[c], in_=hsT[c * 128:(c + 1) * 128, :])
        nc.sync.dma_start(out=ones_sb, in_=hsT[D:D + 1, 0:512])

        def load_w(handle, width, nm):
            tiles = [pp.tile([128, width], BBF, name=f"{nm}{c}", tag=f"{nm}{c}")
                     for c in range(6)]
            brow = pp.tile([1, width], BBF, name=f"{nm}b", tag=f"{nm}b")
            for c in range(6):
                eng = nc.scalar if c % 2 == 0 else nc.sync
                eng.dma_start(out=tiles[c],
                              in_=handle[c * 128:(c + 1) * 128, :])
            nc.scalar.dma_start(out=brow, in_=handle[D:D + 1, :])
            return tiles, brow

        wq_sb, wqb_sb = load_w(wqT, FPC, "wq")
        wk_sb, wkb_sb = load_w(wkT, FPC, "wk")
        wv_sb, wvb_sb = load_w(wvT, HPC * (HD + 1), "wv")

        ow_sb = [pp.tile([128, D], BBF, name=f"ow{c}", tag=f"ow{c}")
                 for c in range(3)]
        for c in range(3):
            nc.scalar.dma_start(out=ow_sb[c], in_=owT[c * 128:(c + 1) * 128, :])
        sel_sb = pp.tile([128, FPC], BBF, name="sel", tag="sel")
        nc.scalar.dma_start(out=sel_sb, in_=selp[:, :])

        # Per-head zero-padded qT/kT [128, S]: rows 0-63 = head features,
        # rows 64-127 = 0.  K=128 scores matmuls keep the PE array fully
        # active (K=64 ran at HAM half-clock) and zeros contribute nothing.
        qTz = [pp.tile([128, S], BBF, name=f"qTz{h}", tag=f"qTz{h}")
               for h in range(HPC)]
        kTz = [pp.tile([128, S], BBF, name=f"kTz{h}", tag=f"kTz{h}")
               for h in range(HPC)]
        # v6pad [128, 6*128]: head h occupies cols h*128..h*128+64 (64 v cols
        # + ones col), cols 65-127 of each block zero -> ctx lhsT is a full
        # [128, 128] stationary.
        v6_sb = [pp.tile([128, HPC * 128], BBF, name=f"v6_{t}", tag=f"v6_{t}")
                 for t in range(ST)]
        # PE warm-up: dummy matmuls on (uninitialized, never-read) data so
        # the HAM clock gate reaches 8/8 while the initial DMAs land.  No
        # deps -> starts right after the preamble.
        warm_sb = pp.tile([128, 512], BBF, name="warm_sb", tag="warm_sb")
        nc.vector.memset(warm_sb, 0.0)
        warm_ps = ps_pool.tile([128, 512], F32, tag="work", name="warm_ps")
        for _ in range(56):
            nc.tensor.matmul(warm_ps, lhsT=warm_sb[:, 0:128], rhs=warm_sb,
                             start=True, stop=True)
        nc.vector.tensor_copy(out=warm_sb[:, 0:1], in_=warm_ps[:, 0:1])

        # Zero only what must be zero, off the DVE critical path:
        # qTz/kTz rows 64-127 on GpSimd (head order, so head 0 unblocks
        # first); v6's 63-wide per-head pad columns via a tiny strided DVE
        # memset.
        for h in range(HPC):
            nc.gpsimd.memset(qTz[h][64:128, :], 0.0)
            nc.gpsimd.memset(kTz[h][64:128, :], 0.0)
        for t in range(ST):
            nc.vector.memset(
                v6_sb[t].rearrange("p (h c) -> p h c", c=128)[:, :, HD + 1:], 0.0)
        ctxu_sb = [pp.tile([65, S], BBF, name=f"ctxu{h}", tag=f"ctxu{h}")
                   for h in range(HPC)]
        ctxa_sb = [pp.tile([128, S], BBF, name=f"ctxa{t}", tag=f"ctxa{t}")
                   for t in range(3)]

        # ---------------- projection emitters ----------------------------
        _prew = [0]

        def _prew_tile(nm):
            # rotate projection psums over 3 slots: the 2 "work" slots plus
            # the (idle until attention) ctx slot -> denser prework PE
            _prew[0] += 1
            if _prew[0] % 3 == 0:
                return ctx_pool.tile([128, 512], F32, tag="ctx", name=nm)
            return ps_pool.tile([128, 512], F32, tag="work", name=nm)

        def qk_chunk(w_tiles, w_brow, dst, t, nb):
            # one [128, 512] psum tile of the q or k projection for feature
            # M-tile t (heads 2t, 2t+1), seq block nb; split-copy the two
            # heads' 64-row halves into their zero-padded dsts.
            ps = _prew_tile(f"qkps{id(dst)}_{t}_{nb}")
            for c in range(6):
                nc.tensor.matmul(
                    ps,
                    lhsT=w_tiles[c][:, t * 128:(t + 1) * 128],
                    rhs=hsT_sb[c][:, nb * 512:(nb + 1) * 512],
                    start=(c == 0), stop=False)
            nc.tensor.matmul(
                ps,
                lhsT=w_brow[:, t * 128:(t + 1) * 128],
                rhs=ones_sb,
                start=False, stop=True)
            ns = slice(nb * 512, (nb + 1) * 512)
            nc.vector.tensor_copy(out=dst[2 * t][0:64, ns], in_=ps[0:64, :])
            nc.scalar.copy(out=dst[2 * t + 1][0:64, ns], in_=ps[64:128, :])

        def v_tile(t):
            VW = HPC * (HD + 1)  # 390
            ps = _prew_tile(f"vps{t}")[:, 0:VW]
            for c in range(6):
                nc.tensor.matmul(
                    ps,
                    lhsT=hsT_sb[c][:, t * 128:(t + 1) * 128],
                    rhs=wv_sb[c],
                    start=(c == 0), stop=False)
            nc.tensor.matmul(
                ps,
                lhsT=ones_sb[:, 0:128],
                rhs=wvb_sb,
                start=False, stop=True)
            # scatter [128, 6, 65] -> cols 0..64 of each 128-wide head block
            nc.vector.tensor_copy(
                out=v6_sb[t].rearrange("p (h c) -> p h c", c=128)[:, :, 0:HD + 1],
                in_=ps.rearrange("p (h c) -> p h c", c=HD + 1))

        # all projections ahead of attention (attention is ACT/PE-balanced,
        # so interleaving prework into it just stretches the exp pipeline)
        for t in range(3):
            for nb in range(4):
                qk_chunk(wq_sb, wqb_sb, qTz, t, nb)
                qk_chunk(wk_sb, wkb_sb, kTz, t, nb)
        for t in range(ST):
            v_tile(t)

        sums6 = pp.tile([HPC, S], F32, name="sums6", tag="sums6")

        # ---------------- attention, head by head -----------------------
        for h in range(HPC):
            ctx_ps = ctx_pool.tile([128, S], F32, tag="ctx", name=f"ctx{h}")
            for kt in range(KT):
                mt = mask_pool.tile([128, S], BBF, tag="m", name=f"m{h}_{kt}")
                dma_eng = nc.sync if kt % 2 == 0 else nc.gpsimd
                dma_eng.dma_start(out=mt, in_=mT[h, kt])
                # emit both q-halves' scores before any ctx so the two exps
                # run back-to-back on ACT (ctx in the PE FIFO between them
                # serialized exp1 behind mult0 <- exp0, ~1.2us/kt)
                sts, ps = [], []
                for qh in range(2):
                    st = ps_pool.tile([128, 1024], F32, tag="work",
                                      name=f"st{h}_{kt}_{qh}")
                    for i in range(2):
                        q0 = qh * 1024 + i * 512
                        nc.tensor.matmul(
                            st[:, i * 512:(i + 1) * 512],
                            lhsT=kTz[h][:, kt * 128:(kt + 1) * 128],
                            rhs=qTz[h][:, q0:q0 + 512],
                            start=True, stop=True)
                    sts.append(st)
                for qh in range(2):
                    p = p_pool.tile([128, 1024], BBF, tag="p",
                                    name=f"p{h}_{kt}_{qh}")
                    nc.scalar.activation(p, sts[qh], EXP,
                                         scale=1.0 / math.sqrt(HD))
                    nc.vector.tensor_tensor(
                        p, p, mt[:, qh * 1024:(qh + 1) * 1024], MULT)
                    ps.append(p)
                for qh in range(2):
                    for i in range(2):
                        q0 = qh * 1024 + i * 512
                        nc.tensor.matmul(
                            ctx_ps[:, q0:q0 + 512],
                            lhsT=v6_sb[kt][:, h * 128:(h + 1) * 128],
                            rhs=ps[qh][:, i * 512:(i + 1) * 512],
                            start=(kt == 0), stop=(kt == KT - 1))
            # head epilogue: ctxT+sums row to SBUF, then gather the bf16
            # sums row into the f32 sums6 row h via a casting SWDGE DMA
            # (no engine cost, any partition).
            nc.vector.tensor_copy(out=ctxu_sb[h], in_=ctx_ps[0:65, :])
            nc.gpsimd.dma_start(out=sums6[h:h + 1, :],
                                in_=ctxu_sb[h][64:65, :])

        # keep the PE array busy through the normalization latency chain so
        # the HAM clock gate doesn't re-throttle before the out-projection
        warm2_ps = ps_pool.tile([128, 512], F32, tag="work", name="warm2_ps")
        for _ in range(48):
            nc.tensor.matmul(warm2_ps, lhsT=warm_sb[:, 0:128], rhs=warm_sb,
                             start=True, stop=True)

        # ---------------- batched softmax normalization ------------------
        # one 6-lane DVE reciprocal, bf16 convert, then per-head selector
        # matmul broadcast (sel has a single 1 per 64-column block) and a
        # normalize multiply
        recipf = pp.tile([HPC, S], F32, name="recipf", tag="recipf")
        nc.vector.reciprocal_approx_fast(out=recipf, in_=sums6)
        recipb = pp.tile([128, S], BBF, name="recipb", tag="recipb")
        nc.vector.memset(recipb, 0.0)
        nc.vector.tensor_copy(out=recipb[0:HPC, :], in_=recipf)
        for h in range(HPC):
            t, ro = h // 2, (h % 2) * 64
            for qh in range(2):
                rb = ps_pool.tile([64, 1024], F32, tag="work",
                                  name=f"rb{h}_{qh}")
                for i in range(2):
                    q0 = qh * 1024 + i * 512
                    nc.tensor.matmul(
                        rb[:, i * 512:(i + 1) * 512],
                        lhsT=sel_sb[:, h * 64:(h + 1) * 64],
                        rhs=recipb[:, q0:q0 + 512],
                        start=True, stop=True)
                nc.vector.tensor_tensor(
                    ctxa_sb[t][ro:ro + 64, qh * 1024:(qh + 1) * 1024],
                    ctxu_sb[h][0:64, qh * 1024:(qh + 1) * 1024],
                    rb, MULT)

        # ---------------- out projection (partial over 384 features) ----
        for si in range(ST):
            if si % 3 == 2:
                op = ctx_pool.tile([128, D], F32, tag="ctx", name=f"ops{si}")
            else:
                op = ps_pool.tile([128, D], F32, tag="work", name=f"ops{si}")
            for c in range(3):
                nc.tensor.matmul(
                    op[:, 0:512],
                    lhsT=ctxa_sb[c][:, si * 128:(si + 1) * 128],
                    rhs=ow_sb[c][:, 0:512],
                    start=(c == 0), stop=(c == 2))
                nc.tensor.matmul(
                    op[:, 512:D],
                    lhsT=ctxa_sb[c][:, si * 128:(si + 1) * 128],
                    rhs=ow_sb[c][:, 512:D],
                    start=(c == 0), stop=(c == 2))
            ot = o_pool.tile([128, D], BBF, tag="o", name=f"ot{si}")
            if si % 2 == 0:
                nc.vector.tensor_copy(out=ot, in_=op)
            else:
                nc.scalar.copy(out=ot, in_=op)
            nc.sync.dma_start(out=out[si * 128:(si + 1) * 128, :], in_=ot)

    return nc


def _get_nc(finalized=False):
    if "nc" not in _CACHE:
        _CACHE["nc"] = _build_bass()
    nc = _CACHE["nc"]
    if finalized and not nc.is_finalized():
        nc.finalize()
    return nc


def _prep_core_inputs(inputs, core):
    """Host-side shard prep for one core: slice + transpose + bf16."""
    hs = np.asarray(inputs["hidden_states"], np.float32)
    mask = np.asarray(inputs["attention_mask"])
    q_w = np.asarray(inputs["q_w"], np.float32)
    q_b = np.asarray(inputs["q_b"], np.float32)
    k_w = np.asarray(inputs["k_w"], np.float32)
    k_b = np.asarray(inputs["k_b"], np.float32)
    v_w = np.asarray(inputs["v_w"], np.float32)
    v_b = np.asarray(inputs["v_b"], np.float32)
    out_w = np.asarray(inputs["out_w"], np.float32)

    b, hh = divmod(core, 2)
    hsl = slice(hh * FPC, (hh + 1) * FPC)

    hsT_aug = np.empty((D + 1, S), np.float32)
    hsT_aug[:D] = hs[b].T
    hsT_aug[D] = 1.0

    def aug_T(w, bias):
        a = np.empty((D + 1, FPC), np.float32)
        a[:D] = w[hsl].T
        a[D] = bias[hsl]
        return a

    wv = np.zeros((D + 1, HPC * (HD + 1)), np.float32)
    for j in range(HPC):
        fs = hh * FPC + j * HD
        wv[0:D, j * (HD + 1):j * (HD + 1) + HD] = v_w[fs:fs + HD].T
        wv[D, j * (HD + 1):j * (HD + 1) + HD] = v_b[fs:fs + HD]
        wv[D, j * (HD + 1) + HD] = 1.0

    sel = np.zeros((128, FPC), np.float32)
    for j in range(HPC):
        sel[j, j * HD:(j + 1) * HD] = 1.0

    heads = slice(hh * HPC, (hh + 1) * HPC)
    mT6 = np.ascontiguousarray(
        mask[0, heads].transpose(0, 2, 1)).reshape(HPC, KT, 128, S)

    return {
        "hsT": hsT_aug.astype(BF16),
        "wqT": aug_T(q_w, q_b).astype(BF16),
        "wkT": aug_T(k_w, k_b).astype(BF16),
        "wvT": wv.astype(BF16),
        "owT": np.ascontiguousarray(out_w[:, hsl].T).astype(BF16),
        "selp": sel.astype(BF16),
        "mT": mT6.astype(BF16),
    }


def kernel(**inputs):
    global _last_result
    nc = _get_nc(finalized=True)
    in_maps = [_prep_core_inputs(inputs, c) for c in range(NCORES)]
    res = run_bass_kernel_spmd(
        nc, in_maps, core_ids=list(range(NCORES)),
        tmpdir=os.environ.get("KERNEL_TRACE_DIR") or None)
    _last_result = res
    outs = [np.asarray(r["out"], dtype=np.float32) for r in res.results]
    out_b = np.asarray(inputs["out_b"], np.float32)
    full = np.empty((B, S, D), np.float32)
    for b in range(B):
        full[b] = outs[2 * b] + outs[2 * b + 1] + out_b
    return full



# revision 9
# speedup vs baseline: 1.0742x; 1.0742x over previous
"""Trainium2 Bass kernel for masked multi-head attention.

Problem: B=4, S=2048, D=768, H=12 (head_dim=64), boolean prune mask per
head, softmax over keys, out-projection.

Sharding (8 cores): data-parallel over batch (4) x tensor-parallel over
head halves (2 x 6 heads).  Core c handles batch c//2 and heads
(c%2)*6 .. (c%2)*6+5.  Host sums the two partial out-projections per
batch and adds out_b.

Design:
  * QKV projections and the out-projection run in bf16 (fp8 weights /
    hidden states cost ~2.5e-2 rel err - over budget).  q/k biases are
    added by DVE tensor_scalar during the PSUM->SBUF fp8 copy; the v
    bias (+ per-head ones column for softmax denominators) is a K=1
    bf16 ones-row matmul.
  * Scores fold the prune mask INTO an fp8 DoubleRow matmul: stationary
    planes (k-features, -240*I), moving planes (q-features, 1-mask).
    -240 is exactly representable in TRN e4m3 and maps to -30 after the
    1/8 softmax scale -> exp rounds to 0 in fp8.  No DVE mask multiply;
    mask DMA is 1 byte/entry streamed per (head, ktile) into plane 1 of
    a rotating qm buffer whose plane 0 holds that head-pair's q.
  * Odd heads live in partitions 64-127 end-to-end (k-features, ctx
    rows, selector bands), so no op ever shifts partitions.
  * exp (ACT) writes p directly as fp8 into (even-kt, odd-kt) planes of
    pP tiles; ctx DoubleRow matmuls then contract 256 key positions per
    step at the same per-column rate (measured 217ns/N=512).  A tunable
    subset of exp tiles runs on DVE via Schraudolph bitcast fast-exp to
    offload the ACT bottleneck.
  * Normalization: denominators via ones-column rows, batched DVE
    reciprocal, selector matmul broadcast (K=6), DVE multiply writing
    bf16 ctxa tiles consumed by the bf16 out-projection.
"""

import os
import sys
import math

import numpy as np

try:
    import concourse.bass as bass
except ImportError:  # pragma: no cover - path fallback for fresh dirs
    for _p in ("/opt/trn_rl_repo", "/root/.axon_site/_ro/trn_rl_repo"):
        if os.path.isdir(_p) and _p not in sys.path:
            sys.path.insert(0, _p)
    import concourse.bass as bass

import ml_dtypes
import concourse.mybir as mybir
from concourse import bacc
from concourse.tile import TileContext
from concourse.bass_utils import run_bass_kernel_spmd

E4 = ml_dtypes.float8_e4m3
BF16 = ml_dtypes.bfloat16
F8 = mybir.dt.float8e4
F32 = mybir.dt.float32
I32 = mybir.dt.int32
BBF = mybir.dt.bfloat16
DR = mybir.MatmulPerfMode.DoubleRow

B, S, D, H = 4, 2048, 768, 12
HD = 64          # head dim
HPC = 6          # heads per core
FPC = HPC * HD   # features per core (384)
VW = HPC * (HD + 1)  # 390
NCORES = 8
KT = S // 128    # 16 key tiles
ST = S // 128    # 16 seq tiles

# (kt, qh) exp tiles offloaded from ACT to DVE fast-exp, per head
OFFLOAD = ((3, 1), (6, 1), (9, 1), (12, 1))
# Schraudolph fast exp: bitcast(int32(x*C + D)) ~= exp(x/8)
FX_C = (2.0 ** 23) * math.log2(math.e) / 8.0
FX_D = (127.0 - 0.05792) * (2.0 ** 23)

_CACHE = {}
_last_result = None


def _build_bass():
    nc = bacc.Bacc()

    hsT = nc.declare_dram_parameter("hsT", [D, S], BBF, isOutput=False)
    wqT = nc.declare_dram_parameter("wqT", [D, FPC], BBF, isOutput=False)
    wkT = nc.declare_dram_parameter("wkT", [D, FPC], BBF, isOutput=False)
    wvT = nc.declare_dram_parameter("wvT", [D, VW], BBF, isOutput=False)
    wvb = nc.declare_dram_parameter("wvb", [1, VW], BBF, isOutput=False)
    qkb = nc.declare_dram_parameter("qkb", [128, 6], F32, isOutput=False)
    owT = nc.declare_dram_parameter("owT", [3, 128, D], BBF, isOutput=False)
    sel2 = nc.declare_dram_parameter("sel2", [6, HPC * 128], BBF, isOutput=False)
    dgI = nc.declare_dram_parameter("dgI", [128, S], F8, isOutput=False)
    maskP = nc.declare_dram_parameter("maskP", [HPC, KT, 128, S], F8,
                                      isOutput=False)
    out = nc.declare_dram_parameter("out", [S, D], BBF, isOutput=True)

    EXP = mybir.ActivationFunctionType.Exp
    MULT = mybir.AluOpType.mult
    ADD = mybir.AluOpType.add

    with TileContext(nc) as tc, \
            tc.tile_pool(name="persist", bufs=1) as pp, \
            tc.tile_pool(name="qmp", bufs=3) as qm_pool, \
            tc.tile_pool(name="pbuf", bufs=3) as p_pool, \
            tc.tile_pool(name="fxp", bufs=2) as fx_pool, \
            tc.tile_pool(name="obuf", bufs=2) as o_pool, \
            tc.tile_pool(name="pswork", bufs=2, space="PSUM") as ps_pool, \
            tc.tile_pool(name="psctx", bufs=1, space="PSUM") as ctx_pool:

        # ---------------- persistent SBUF tensors + input DMAs ----------
        hsT_sb = [pp.tile([128, S], BBF, name=f"hsT{c}", tag=f"hsT{c}")
                  for c in range(6)]
        for c in range(6):
            eng = (nc.sync, nc.scalar, nc.gpsimd)[c % 3]
            eng.dma_start(out=hsT_sb[c], in_=hsT[c * 128:(c + 1) * 128, :])

        def load_w(handle, width, nm, eng):
            tiles = [pp.tile([128, width], BBF, name=f"{nm}{c}",
                             tag=f"{nm}{c}") for c in range(6)]
            for c in range(6):
                eng.dma_start(out=tiles[c],
                              in_=handle[c * 128:(c + 1) * 128, :])
            return tiles

        wq_sb = load_w(wqT, FPC, "wq", nc.scalar)
        wk_sb = load_w(wkT, FPC, "wk", nc.sync)
        wv_sb = load_w(wvT, VW, "wv", nc.gpsimd)
        wvb_sb = pp.tile([1, VW], BBF, name="wvb_sb", tag="wvb_sb")
        nc.scalar.dma_start(out=wvb_sb, in_=wvb[:, :])
        qkb_sb = pp.tile([128, 6], F32, name="qkb_sb", tag="qkb_sb")
        nc.scalar.dma_start(out=qkb_sb, in_=qkb[:, :])
        ow_sb = [pp.tile([128, D], BBF, name=f"ow{c}", tag=f"ow{c}")
                 for c in range(3)]
        for c in range(3):
            nc.sync.dma_start(out=ow_sb[c], in_=owT[c])
        sel2_sb = pp.tile([6, HPC * 128], BBF, name="sel2_sb", tag="sel2_sb")
        nc.scalar.dma_start(out=sel2_sb, in_=sel2[:, :])

        # per-head score stationaries [128, 2, S]: plane0 = k-features
        # (rows (h%2)*64..+63; other rows zero), plane1 = -240*I tiled
        kS = [pp.tile([128, 2, S], F8, name=f"kS{h}", tag=f"kS{h}")
              for h in range(HPC)]
        for h in range(HPC):
            eng = (nc.sync, nc.gpsimd, nc.scalar)[h % 3]
            eng.dma_start(out=kS[h][:, 1, :], in_=dgI[:, :])

        ones_sb = pp.tile([1, 512], BBF, name="ones_sb", tag="ones_sb")
        nc.vector.memset(ones_sb, 1.0)

        # PE warm-up while input DMAs land (HAM clock gate)
        warm_sb = pp.tile([128, 512], BBF, name="warm_sb", tag="warm_sb")
        nc.vector.memset(warm_sb, 0.0)
        warm_ps = ps_pool.tile([128, 512], F32, tag="work", name="warm_ps")
        for _ in range(40):
            nc.tensor.matmul(warm_ps, lhsT=warm_sb[:, 0:128], rhs=warm_sb,
                             start=True, stop=True)
        nc.vector.tensor_copy(out=warm_sb[:, 0:1], in_=warm_ps[:, 0:1])
        # trigger the exp table load early (off the critical path)
        exp_pre = pp.tile([1, 16], F32, name="exp_pre", tag="exp_pre")
        nc.scalar.activation(exp_pre, warm_ps[0:1, 0:16], EXP)

        # zero fills (gpsimd, off the DVE critical path)
        for h in range(HPC):
            r = slice(64, 128) if h % 2 == 0 else slice(0, 64)
            nc.gpsimd.memset(kS[h][r, 0, :], 0.0)

        qT8 = [pp.tile([128, S], F8, name=f"qT8{t}", tag=f"qT8{t}")
               for t in range(3)]
        # v stationaries per kt-pair: [128, 2, 6*128], plane = kt parity.
        # Even heads: v cols 0-63 + ones col 64; odd heads: ones col 63 +
        # v cols 64-127 (so odd heads' ctx lands on partitions 64-127).
        v6P = [pp.tile([128, 2, HPC * 128], F8, name=f"v6P{i}", tag=f"v6P{i}")
               for i in range(8)]
        for i in range(8):
            v4 = v6P[i].rearrange("p t (h c) -> p t h c", c=128)
            nc.gpsimd.memset(v4[:, :, 0::2, HD + 1:], 0.0)
            nc.gpsimd.memset(v4[:, :, 1::2, 0:HD - 1], 0.0)

        ctxu = [pp.tile([128, S], BBF, name=f"ctxu{h}", tag=f"ctxu{h}")
                for h in range(HPC)]
        ctxa = [pp.tile([128, S], BBF, name=f"ctxa{t}", tag=f"ctxa{t}")
                for t in range(3)]
        sums6 = pp.tile([HPC, S], F32, name="sums6", tag="sums6")

        # ---------------- projection emitters ----------------------------
        # prework psums use the "work" slots: tiles interleaved inside
        # head 0 must never wait on the ctx slot (deadlock via v_tile ->
        # ctx dependency).
        def qk_chunk(w_sb, is_q, t, nb):
            ps = ps_pool.tile([128, 512], F32, tag="work",
                              name=f"qk{int(is_q)}_{t}_{nb}")
            for c in range(6):
                nc.tensor.matmul(
                    ps,
                    lhsT=w_sb[c][:, t * 128:(t + 1) * 128],
                    rhs=hsT_sb[c][:, nb * 512:(nb + 1) * 512],
                    start=(c == 0), stop=(c == 5))
            ns = slice(nb * 512, (nb + 1) * 512)
            if is_q:
                nc.vector.tensor_scalar(qT8[t][:, ns], ps,
                                        qkb_sb[:, t:t + 1], None, ADD)
            else:
                nc.vector.tensor_scalar(kS[2 * t][0:64, 0, ns], ps[0:64],
                                        qkb_sb[0:64, 3 + t:4 + t], None, ADD)
                nc.vector.tensor_scalar(kS[2 * t + 1][64:128, 0, ns],
                                        ps[64:128],
                                        qkb_sb[64:128, 3 + t:4 + t], None, ADD)

        def v_tile(t):
            ps = ps_pool.tile([128, VW], F32, tag="work", name=f"vps{t}")
            for c in range(6):
                nc.tensor.matmul(
                    ps,
                    lhsT=hsT_sb[c][:, t * 128:(t + 1) * 128],
                    rhs=wv_sb[c],
                    start=(c == 0), stop=False)
            nc.tensor.matmul(ps, lhsT=ones_sb[:, 0:128], rhs=wvb_sb,
                             start=False, stop=True)
            dst = v6P[t // 2][:, t % 2, :].rearrange("p (h c) -> p h c", c=128)
            src = ps.rearrange("p (h c) -> p h c", c=HD + 1)
            nc.vector.tensor_copy(out=dst[:, 0::2, 0:HD + 1], in_=src[:, 0::2])
            nc.vector.tensor_copy(out=dst[:, 1::2, HD - 1:128], in_=src[:, 1::2])

        # ---------------- attention, head by head -----------------------
        # prework is interleaved into head 0 so PE slack absorbs it while
        # ACT streams exp.
        for nb in range(4):
            qk_chunk(wq_sb, True, 0, nb)
            qk_chunk(wk_sb, False, 0, nb)
        v_tile(0)
        v_tile(1)

        def head(h, qm_bufs):
            ctx_ps = ctx_pool.tile([128, S], F32, tag="ctx", name=f"ctx{h}")
            pP_cur = [None]
            for kt in range(KT):
                if h % 2 == 0 and kt < 3:
                    qm = qm_pool.tile([128, 2, S], F8, tag="qm",
                                      name=f"qm{h}_{kt}")
                    nc.vector.tensor_copy(out=qm[:, 0, :], in_=qT8[h // 2])
                    qm_bufs[kt] = qm
                else:
                    qm = qm_bufs[kt % 3]
                dma_eng = nc.sync if kt % 2 == 0 else nc.gpsimd
                dma_eng.dma_start(out=qm[:, 1, :], in_=maskP[h, kt])
                if kt % 2 == 0:
                    pP_cur[0] = p_pool.tile([128, 2, S], F8, tag="p",
                                            name=f"p{h}_{kt}")
                pP = pP_cur[0]
                sts = []
                for qh in range(2):
                    st = ps_pool.tile([128, 1024], F32, tag="work",
                                      name=f"st{h}_{kt}_{qh}")
                    for i in range(2):
                        q0 = qh * 1024 + i * 512
                        nc.tensor.matmul(
                            st[:, i * 512:(i + 1) * 512],
                            lhsT=kS[h][:, :, kt * 128:(kt + 1) * 128],
                            rhs=qm[:, :, q0:q0 + 512],
                            start=True, stop=True, perf_mode=DR)
                    sts.append(st)
                for qh in range(2):
                    dst = pP[:, kt % 2, qh * 1024:(qh + 1) * 1024]
                    if (kt, qh) in OFFLOAD:
                        fx = fx_pool.tile([128, 1024], I32, tag="fx",
                                          name=f"fx{h}_{kt}_{qh}")
                        nc.vector.tensor_scalar(fx, sts[qh], FX_C, FX_D,
                                                MULT, ADD)
                        nc.vector.tensor_copy(out=dst, in_=fx.bitcast(F32))
                    else:
                        nc.scalar.activation(dst, sts[qh], EXP,
                                             scale=1.0 / math.sqrt(HD))
                if kt % 2 == 1:
                    i = kt // 2
                    if h == 0:
                        if i < 7:
                            v_tile(2 * i + 2)
                            v_tile(2 * i + 3)
                        if i == 2:
                            for nb in range(4):
                                qk_chunk(wq_sb, True, 1, nb)
                                qk_chunk(wk_sb, False, 1, nb)
                        if i == 5:
                            for nb in range(4):
                                qk_chunk(wq_sb, True, 2, nb)
                                qk_chunk(wk_sb, False, 2, nb)
                    for j in range(4):
                        q0 = j * 512
                        nc.tensor.matmul(
                            ctx_ps[:, q0:q0 + 512],
                            lhsT=v6P[i][:, :, h * 128:(h + 1) * 128],
                            rhs=pP[:, :, q0:q0 + 512],
                            start=(i == 0), stop=(i == 7), perf_mode=DR)
            # head epilogue: ctx rows + denominator row to SBUF, then the
            # f32 sums row via a casting SWDGE DMA (no engine cost).
            if h % 2 == 0:
                nc.vector.tensor_copy(out=ctxu[h][0:HD + 1, :],
                                      in_=ctx_ps[0:HD + 1, :])
                nc.gpsimd.dma_start(out=sums6[h:h + 1, :],
                                    in_=ctxu[h][HD:HD + 1, :])
            else:
                # partition-base rule: <=32 partitions from base 32, 64
                # from base 64 -> two copies (row 63 = denominator)
                nc.vector.tensor_copy(out=ctxu[h][32:64, :],
                                      in_=ctx_ps[32:64, :])
                nc.vector.tensor_copy(out=ctxu[h][64:128, :],
                                      in_=ctx_ps[64:128, :])
                nc.gpsimd.dma_start(out=sums6[h:h + 1, :],
                                    in_=ctxu[h][HD - 1:HD, :])

        qm_bufs = [None, None, None]
        for h in range(HPC):
            head(h, qm_bufs)

        # keep the PE array busy through the normalization latency chain
        warm2_ps = ps_pool.tile([128, 512], F32, tag="work", name="warm2_ps")
        for _ in range(16):
            nc.tensor.matmul(warm2_ps, lhsT=warm_sb[:, 0:128], rhs=warm_sb,
                             start=True, stop=True)

        # ---------------- batched softmax normalization ------------------
        recipf = pp.tile([HPC, S], F32, name="recipf", tag="recipf")
        nc.vector.reciprocal_approx_fast(out=recipf, in_=sums6)
        recipb = pp.tile([HPC, S], BBF, name="recipb", tag="recipb")
        nc.vector.tensor_copy(out=recipb, in_=recipf)
        for h in range(HPC):
            ro = (h % 2) * 64
            for qh in range(2):
                rb = ps_pool.tile([128, 1024], F32, tag="work",
                                  name=f"rb{h}_{qh}")
                for i in range(2):
                    q0 = qh * 1024 + i * 512
                    nc.tensor.matmul(
                        rb[:, i * 512:(i + 1) * 512],
                        lhsT=sel2_sb[:, h * 128:(h + 1) * 128],
                        rhs=recipb[:, q0:q0 + 512],
                        start=True, stop=True)
                nc.vector.tensor_tensor(
                    ctxa[h // 2][ro:ro + 64, qh * 1024:(qh + 1) * 1024],
                    ctxu[h][ro:ro + 64, qh * 1024:(qh + 1) * 1024],
                    rb[ro:ro + 64, :], MULT)

        # ---------------- out projection (partial over 384 features) ----
        for si in range(ST):
            if si % 3 == 2:
                op = ctx_pool.tile([128, 1024], F32, tag="ctx", name=f"o{si}")
            else:
                op = ps_pool.tile([128, 1024], F32, tag="work", name=f"o{si}")
            for c in range(3):
                nc.tensor.matmul(
                    op[:, 0:512],
                    lhsT=ctxa[c][:, si * 128:(si + 1) * 128],
                    rhs=ow_sb[c][:, 0:512],
                    start=(c == 0), stop=(c == 2))
                nc.tensor.matmul(
                    op[:, 512:D],
                    lhsT=ctxa[c][:, si * 128:(si + 1) * 128],
                    rhs=ow_sb[c][:, 512:D],
                    start=(c == 0), stop=(c == 2))
            ot = o_pool.tile([128, D], BBF, tag="o", name=f"ot{si}")
            if si % 2 == 0:
                nc.vector.tensor_copy(out=ot, in_=op[:, 0:D])
            else:
                nc.scalar.copy(out=ot, in_=op[:, 0:D])
            nc.sync.dma_start(out=out[si * 128:(si + 1) * 128, :], in_=ot)

    return nc


def _get_nc(finalized=False):
    if "nc" not in _CACHE:
        _CACHE["nc"] = _build_bass()
    nc = _CACHE["nc"]
    if finalized and not nc.is_finalized():
        nc.finalize()
    return nc


def _prep_core_inputs(inputs, core):
    """Host-side shard prep for one core."""
    hs = np.asarray(inputs["hidden_states"], np.float32)
    mask = np.asarray(inputs["attention_mask"])
    q_w = np.asarray(inputs["q_w"], np.float32)
    q_b = np.asarray(inputs["q_b"], np.float32)
    k_w = np.asarray(inputs["k_w"], np.float32)
    k_b = np.asarray(inputs["k_b"], np.float32)
    v_w = np.asarray(inputs["v_w"], np.float32)
    v_b = np.asarray(inputs["v_b"], np.float32)
    out_w = np.asarray(inputs["out_w"], np.float32)

    b, hh = divmod(core, 2)
    hsl = slice(hh * FPC, (hh + 1) * FPC)

    # per-head 65-wide v blocks; even heads (v0..v63, ones) -> ctx rows
    # 0-63 + denom row 64; odd heads (ones, v0..v63) so the on-chip
    # scatter to cols 63..127 puts ones/denom at row 63, v at 64-127.
    wv65 = np.zeros((D, VW), np.float32)
    wvbv = np.zeros((1, VW), np.float32)
    for j in range(HPC):
        fs = hh * FPC + j * HD
        off = j * (HD + 1) + (j % 2)
        wv65[:, off:off + HD] = v_w[fs:fs + HD].T
        wvbv[0, off:off + HD] = v_b[fs:fs + HD]
        wvbv[0, j * (HD + 1) + (0 if j % 2 else HD)] = 1.0

    qkbv = np.empty((128, 6), np.float32)
    qkbv[:, 0:3] = q_b[hsl].reshape(3, 128).T
    qkbv[:, 3:6] = k_b[hsl].reshape(3, 128).T

    sel2v = np.zeros((6, HPC * 128), np.float32)
    for h in range(HPC):
        lo = (h % 2) * 64
        sel2v[h, h * 128 + lo:h * 128 + lo + 64] = 1.0

    dgIv = np.tile(-240.0 * np.eye(128, dtype=np.float32), (1, KT)).astype(E4)

    heads = slice(hh * HPC, (hh + 1) * HPC)
    mT6 = np.ascontiguousarray(mask[0, heads].transpose(0, 2, 1))
    # (1 - m) as fp8 bytes: 1.0 -> 0x38, 0.0 -> 0x00
    mP = np.where(mT6, np.uint8(0), np.uint8(0x38)).reshape(HPC, KT, 128, S)

    return {
        "hsT": np.ascontiguousarray(hs[b].T).astype(BF16),
        "wqT": np.ascontiguousarray(q_w[hsl].T).astype(BF16),
        "wkT": np.ascontiguousarray(k_w[hsl].T).astype(BF16),
        "wvT": wv65.astype(BF16),
        "wvb": wvbv.astype(BF16), "qkb": qkbv,
        "owT": np.ascontiguousarray(
            out_w.T[hsl].reshape(3, 128, D)).astype(BF16),
        "sel2": sel2v.astype(BF16),
        "dgI": dgIv, "maskP": mP.view(E4),
    }


def kernel(**inputs):
    global _last_result
    nc = _get_nc(finalized=True)
    in_maps = [_prep_core_inputs(inputs, c) for c in range(NCORES)]
    res = run_bass_kernel_spmd(
        nc, in_maps, core_ids=list(range(NCORES)),
        tmpdir=os.environ.get("KERNEL_TRACE_DIR") or None)
    _last_result = res
    outs = [np.asarray(r["out"], dtype=np.float32) for r in res.results]
    out_b = np.asarray(inputs["out_b"], np.float32)
    full = np.empty((B, S, D), np.float32)
    for b in range(B):
        full[b] = outs[2 * b] + outs[2 * b + 1] + out_b
    return full
